# revision 78
# baseline (speedup 1.0000x reference)
"""Trainium2 Bass kernel for the vq_codebook CCE loss.

Live dataflow of the reference:
    t   = (1/(B*F)) * sum_b min_p ||outputs[b] - clusters[tc_b, p]||^2
    out = ALPHA*t + BETA*(1 - t)
Only the TARGET class's prototype distances feed the loss (the wrong-class
branch of the reference is dead code), so per batch row only 32 of the
6400 prototype distances are live.

Strategy (8 NeuronCores, SPMD):
  - Host sorts rows by target class (stable) and splits the sorted batch
    into 16 tiles of 128 rows.  Each tile's rows span a small contiguous
    class range (<=16 classes for random data), so a single 512-column
    PSUM bank holds every prototype column any of its rows needs.
  - Window width is the ACTUAL max tile span (nwc classes, 15 for the
    reference data), not a padded 16 - every matmul streams and every DVE
    min reads 32*nwc columns.
  - Each core takes 2 tiles; their prototype gathers are packed into one
    class-range union (global width u32, per-core shift) so overlapping
    classes are DMA'd once: tile A reads union cols [0:32*nwc], tile B
    reads [(u32-nwc)*32:].  Falls back to duplicated windows if infeasible.
  - Per tile: 3 fp8 DoubleRow matmuls (256 contraction rows each) compute
    -2*x.c, then one rank-20 DoubleRow matmul adds ||c||^2 (split 16*h+r,
    fp8, abs err <= 2) AND a +224*224 shift on every column outside the
    row's own class window (the one-hot select mask is rank-16, so it
    rides the same matmul; the shift cancels exactly on the own window).
  - A single full-row DVE min per tile yields each row's selected
    nearest-prototype distance directly - no mask/select stage.  Tile A's
    accumulation closes first so its min overlaps tile B's matmuls.
  - ||x||^2 comes from one Scalar-engine Square pass with accum_out over
    the core's fp8 x slice (a = -2x, so sum a^2 = 4 sum x^2).
  - A final f32 ones-matmul reduces [128,3] partials across partitions to
    [1,3] so the output DMA is a single descriptor.
  - 7 dummy 512-col matmuls at kernel start release the PE's HAM clock
    gate (cold PE runs at 1.2 GHz) under the DMA shadow.
  - Host combines: t = (sum x2 + sum selected_min)/(B*F).

Measured: ~18.3-20us HW exec (baseline all-class kernel: 61.3us).
fp8 e4m3 quantization moves t by ~0.03% (validated off-device vs f64).
NOTE: tensor_tensor_reduce crashes the exec unit on this HW (bisected);
do not reintroduce it.  DMA descriptor-gen (DIRECT2D) costs ~0.6-0.8us
per dma_start and scales with partition rows, not bytes - keep DMA count
low and the final output to a single descriptor.
"""

import os
import numpy as np
import ml_dtypes  # noqa: F401  (np dtype registry for bf16/fp8)
from contextlib import ExitStack

import concourse.tile as tile
from concourse import bacc, mybir
from concourse.bass_utils import run_bass_kernel_spmd

ALPHA = 5.0
BETA = 5.0

B, F, C, P = 2048, 768, 200, 32
NCORES = 8
NT = B // 128            # 16 row tiles of 128 sorted rows
TPC = NT // NCORES       # 2 tiles per core
K3 = F // 256            # 3 DoubleRow contraction chunks
RPC = 128 * TPC          # 256 rows per core
BIG = 224.0              # BIG*BIG = 50176 shift for non-selected columns

F32 = mybir.dt.float32
BF16 = mybir.dt.bfloat16
KDT = mybir.dt.float8e4
AX = mybir.AxisListType
OP = mybir.AluOpType

V_DMA = os.environ.get("KV_DMA", "3")  # cg stream pieces: 1 | 3
V_WU = int(os.environ.get("KV_WU", "7"))   # PE warm-up dummy matmuls
V_OUT = os.environ.get("KV_OUT", "pe")  # pe | gps

_prog_cache = {}


def _build_program(nb, nwc, u32):
    """nb = PSUM banks per tile (1 unless some tile spans >16 classes).
    nwc = classes per tile window (the max tile span, <= 16*nb).
    u32 = class slots in the per-core union gather (2*nwc = fully
    duplicated per-tile blocks; less = windows overlap/share columns)."""
    key = ("nc", nb, nwc, u32, V_DMA, V_WU, V_OUT)
    if key in _prog_cache:
        return _prog_cache[key]

    ncol = 32 * nwc          # prototype columns per tile window
    ucol = u32 * 32          # columns in the union gather per core
    offb = ucol - ncol       # tile B's window offset into the union
    mbr = TPC * 2 * ncol     # rhs part of mb
    mbl = TPC * nb * 256     # lhsT part of mb

    nc = bacc.Bacc(
        "TRN2", target_bir_lowering=False, debug=False, num_devices=NCORES,
        enable_asserts=False, enable_partition_id=False,
    )

    xa = nc.dram_tensor("xa", [128, K3 * 2 * RPC], KDT, kind="ExternalInput").ap()
    cg = nc.dram_tensor("cg", [128, K3 * 2 * ucol], KDT, kind="ExternalInput").ap()
    mb = nc.dram_tensor("mb", [10, mbr + mbl], KDT, kind="ExternalInput").ap()
    out = nc.dram_tensor("out", [1, 3], F32, kind="ExternalOutput").ap()

    DR = mybir.MatmulPerfMode.DoubleRow

    with tile.TileContext(nc) as tc, ExitStack() as ctx:
        const = ctx.enter_context(tc.tile_pool(name="const", bufs=1))
        psum = ctx.enter_context(tc.tile_pool(name="psum", bufs=2 * nb, space="PSUM"))
        psco = ctx.enter_context(tc.tile_pool(name="psco", bufs=1, space="PSUM"))

        xa_sb = const.tile([128, K3 * 2 * RPC], KDT, name="xa_sb", tag="xa")
        cg_sb = const.tile([128, K3 * 2 * ucol], KDT, name="cg_sb", tag="cg")
        mb_sb = const.tile([10, mbr + mbl], KDT, name="mb_sb", tag="mb")
        sq = const.tile([128, K3 * 2 * RPC], BF16, name="sq", tag="sq")
        res = const.tile([128, 3], F32, name="res", tag="res")
        ones = const.tile([128, 1], F32, name="ones", tag="on")
        outs = const.tile([1, 3], F32, name="outs", tag="os")

        xa_v = xa_sb[:].rearrange("p (k s r) -> p k s r", k=K3, s=2)
        cg_v = cg_sb[:].rearrange("p (k s j) -> p k s j", k=K3, s=2)
        mbr_v = mb_sb[:, 0:mbr].rearrange("p (t s j) -> p t s j", t=TPC, s=2)
        mbl_v = mb_sb[:, mbr:].rearrange("p (t b s r) -> p t b s r", t=TPC, b=nb, s=2)

        # --- PE warm-up: dummy matmuls release the HAM clock gate (PE runs
        # at 1.2 GHz until ~3.4us of sustained activity) while the DMAs
        # stream, so the real matmuls run at 2.4 GHz. ---
        if V_WU:
            dum = const.tile([128, 512], BF16, name="dum", tag="dum")
            onesb = const.tile([128, 1], BF16, name="onesb", tag="ob")
            psd = psco.tile([1, 512], F32, name="psd", tag="psd")
            nc.gpsimd.memset(dum[:], 0.0)
            nc.gpsimd.memset(onesb[:], 1.0)
            for _ in range(V_WU):
                nc.tensor.matmul(
                    psd[:], lhsT=onesb[:], rhs=dum[:], start=True, stop=True
                )

        # --- DMAs: no dep chains; cg streams on the sync HWDGE ring in
        # chunk order while xa + mb ride the scalar HWDGE ring in parallel.
        # (NB: descriptor-gen time scales with descriptor count = partition
        # rows, so splitting a [128,*] DMA only adds issue latency.) ---
        if V_DMA == "1":
            nc.sync.dma_start(cg_sb[:], cg)
            nc.scalar.dma_start(xa_sb[:], xa)
            nc.scalar.dma_start(mb_sb[:], mb)
        elif V_DMA == "s2":
            # chunks split across both HWDGE rings so all pieces start
            # streaming within one issue slot of the body start
            cg_f = cg_sb[:].rearrange("p (k x) -> p k x", k=K3)
            cg_d = cg.rearrange("p (k x) -> p k x", k=K3)
            nc.sync.dma_start(cg_f[:, 0, :], cg_d[:, 0, :])
            nc.scalar.dma_start(xa_sb[:], xa)
            nc.sync.dma_start(cg_f[:, 2, :], cg_d[:, 2, :])
            nc.scalar.dma_start(cg_f[:, 1, :], cg_d[:, 1, :])
            nc.scalar.dma_start(mb_sb[:], mb)
        else:
            cg_f = cg_sb[:].rearrange("p (k x) -> p k x", k=K3)
            cg_d = cg.rearrange("p (k x) -> p k x", k=K3)
            nc.sync.dma_start(cg_f[:, 0, :], cg_d[:, 0, :])
            nc.sync.dma_start(cg_f[:, 1, :], cg_d[:, 1, :])
            # chunk 2 splits at the tile-A window edge: tile A's closing
            # matmul (and the serial DVE min chain behind it) gets its
            # columns one DMA-receipt window earlier than tile B's tail
            cgs = cg_sb[:].rearrange("p (k s j) -> p k s j", k=K3, s=2)
            cgd = cg.rearrange("p (k s j) -> p k s j", k=K3, s=2)
            nc.sync.dma_start(cgs[:, 2, :, 0:ncol], cgd[:, 2, :, 0:ncol])
            nc.sync.dma_start(cgs[:, 2, :, ncol:], cgd[:, 2, :, ncol:])
            nc.scalar.dma_start(xa_sb[:], xa)
            nc.scalar.dma_start(mb_sb[:], mb)

        if V_OUT == "pe":
            nc.gpsimd.memset(ones[:], 1.0)

        # --- sum x^2 on the Scalar engine in the DMA/PE shadow ---
        nc.scalar.activation(
            out=sq[:], in_=xa_sb[:],
            func=mybir.ActivationFunctionType.Square,
            accum_out=res[:, 0:1],
        )

        # --- PE: per tile, 2 DoubleRow chunks, the rank-20 c2+select
        # matmul (needs only mb), then the last chunk with stop=True so
        # the DVE min fires the moment it retires. ---
        pss = [psum.tile([128, ncol], F32, name="ps", tag="ps") for _ in range(TPC)]

        def mm_chunk(t, k, start, stop):
            off = 0 if t == 0 else offb
            for b in range(nb):
                lo, hi = b * 512, min(ncol, (b + 1) * 512)
                nc.tensor.matmul(
                    pss[t][:, lo:hi],
                    lhsT=xa_v[:, k, :, t * 128 : (t + 1) * 128],
                    rhs=cg_v[:, k, :, off + lo : off + hi],
                    perf_mode=DR,
                    start=start,
                    stop=stop,
                )

        def mm_sel(t):
            for b in range(nb):
                lo, hi = b * 512, min(ncol, (b + 1) * 512)
                nc.tensor.matmul(
                    pss[t][:, lo:hi],
                    lhsT=mbl_v[:, t, b, :, :],
                    rhs=mbr_v[:, t, :, lo:hi],
                    perf_mode=DR,
                    start=False,
                    stop=False,
                )

        for k in range(K3 - 1):
            for t in range(TPC):
                mm_chunk(t, k, start=(k == 0), stop=False)
        # close tile A's accumulation as early as possible: its min then
        # overlaps tile B's remaining matmuls on the DVE
        for t in range(TPC):
            mm_sel(t)
            mm_chunk(t, K3 - 1, start=False, stop=True)

        # --- one full-row min per tile = the selected distance ---
        for t in range(TPC):
            nc.vector.tensor_reduce(
                out=res[:, 1 + t : 2 + t],
                in_=pss[t][:],
                axis=AX.X,
                op=OP.min,
            )

        if V_OUT == "pe":
            # cross-partition reduce on the PE, then a 1-descriptor DMA
            # two reduces: x2+minA fold across partitions while tile B's
            # min is still running; only the 1-col minB reduce sits on the
            # critical tail
            pco = psco.tile([1, 3], F32, name="pco", tag="pco")
            nc.tensor.matmul(
                pco[:, 0:2], lhsT=ones[:], rhs=res[:, 0:2], start=True, stop=True
            )
            nc.tensor.matmul(
                pco[:, 2:3], lhsT=ones[:], rhs=res[:, 2:3], start=True, stop=True
            )
            nc.vector.tensor_scalar_add(
                out=outs[:, 0:2], in0=pco[:, 0:2], scalar1=0.0
            )
            nc.vector.tensor_scalar_add(
                out=outs[:, 2:3], in0=pco[:, 2:3], scalar1=0.0
            )
            nc.sync.dma_start(out, outs[:])
        else:
            # cross-partition reduce on GpSimd straight to SBUF: one engine
            # hop fewer than PE-reduce + copy, still a 1-descriptor DMA
            nc.gpsimd.tensor_reduce(
                out=outs[:], in_=res[:], axis=AX.C, op=OP.add
            )
            nc.sync.dma_start(out, outs[:])

    nc.compile()
    _prog_cache[key] = nc
    return nc


def _prep_inputs(outputs, clusters, target_classes):
    outputs = np.ascontiguousarray(np.asarray(outputs, dtype=np.float32))
    clusters = np.ascontiguousarray(np.asarray(clusters, dtype=np.float32))
    tc_np = np.asarray(target_classes).astype(np.int64)

    np_k = mybir.dt.np(KDT)

    order = np.argsort(tc_np, kind="stable")
    xs = outputs[order]
    tcs = tc_np[order]

    los = np.empty(NT, np.int64)
    his = np.empty(NT, np.int64)
    for t in range(NT):
        seg = tcs[t * 128 : (t + 1) * 128]
        los[t] = seg.min()
        his[t] = seg.max()
    spans = his - los + 1
    nb = max(1, int(-(-int(spans.max()) // 16)))
    nwc = int(spans.max()) if nb == 1 else 16 * nb
    ncol = 32 * nwc
    nw = nwc
    mbr = TPC * 2 * ncol
    mbl = TPC * nb * 256

    # Per-core union layout: tile A's 16-class window sits at union slots
    # [0,16), tile B's at [u32-16, u32); overlapping slots are shared.
    # u32 is global (SPMD); s_c is the per-core placement shift.  Falls
    # back to fully duplicated windows (u32 = 32*nb) when infeasible.
    base = np.empty((NCORES, TPC), np.int64)
    u32 = 2 * nwc
    if nb == 1:
        d = np.array([los[2 * c + 1] - los[2 * c] for c in range(NCORES)])
        sa = np.array([spans[2 * c] for c in range(NCORES)])
        uu = np.array([his[2 * c + 1] - los[2 * c] + 1 for c in range(NCORES)])
        for cand in range(max(nwc, int(uu.max())), 2 * nwc):
            s = np.maximum(0, cand - nwc - d)
            if np.all(s <= np.minimum(nwc - sa, cand - uu)):
                u32 = cand
                for c in range(NCORES):
                    base[c, 0] = los[2 * c] - s[c]
                    base[c, 1] = base[c, 0] + u32 - nwc
                break
    if u32 == 2 * nwc:
        for c in range(NCORES):
            base[c, 0] = los[2 * c]
            base[c, 1] = los[2 * c + 1]
    ucol = u32 * 32
    offs = [0, ucol - ncol]

    flat = clusters.reshape(C * P, F)
    c2 = (flat.astype(np.float64) ** 2).sum(axis=1).astype(np.float32)

    # -2x in fp8, laid out (p, k, s, r): feature = k*256 + s*128 + p
    a8 = np.clip(-2.0 * xs, -240, 240).astype(np_k)  # [B, F]

    big8 = np.float32(BIG).astype(np_k)

    in_maps = []
    for ci in range(NCORES):
        rows = slice(ci * RPC, (ci + 1) * RPC)
        xa_i = np.ascontiguousarray(
            a8[rows].T.reshape(K3, 2, 128, RPC).transpose(2, 0, 1, 3)
            .reshape(128, K3 * 2 * RPC)
        )

        cg_i = np.zeros((128, K3, 2, ucol), np_k)
        mb_i = np.zeros((10, mbr + mbl), np_k)
        mbr_v = mb_i[:, 0:mbr].reshape(10, TPC, 2, ncol)
        mbl_v = mb_i[:, mbr:].reshape(10, TPC, nb, 2, 128)
        for tt in range(TPC):
            t = ci * TPC + tt
            bs = int(base[ci, tt])
            cw = bs + np.arange(nw)                         # class per slot
            valid = (cw >= 0) & (cw < C)
            G = np.zeros((nw * P, F), np.float32)
            for w in np.nonzero(valid)[0]:
                G[w * P : (w + 1) * P] = flat[cw[w] * P : (cw[w] + 1) * P]
            g8 = np.clip(G, -240, 240).astype(np_k)
            # (F, ncol) -> (k, s, p, ncol) -> (p, k, s, ncol)
            cg_i[:, :, :, offs[tt] : offs[tt] + ncol] = (
                g8.T.reshape(K3, 2, 128, nw * P).transpose(2, 0, 1, 3)
            )
            c2t = np.zeros(ncol, np.float32)
            for w in np.nonzero(valid)[0]:
                c2t[w * P : (w + 1) * P] = c2[cw[w] * P : (cw[w] + 1) * P]
            h8 = np.clip(c2t / 16.0, -240, 240).astype(np_k)
            r8 = np.clip(c2t - 16.0 * h8.astype(np.float32), -240, 240).astype(np_k)
            # rhs components (partition kk, slot s) = comp 2*kk+s:
            #   0: h, 1: r, 2: +BIG const, 3+w: -BIG on window w's columns
            mbr_v[0, tt, 0, :] = h8
            mbr_v[0, tt, 1, :] = r8
            mbr_v[1, tt, 0, :] = big8
            wincol = np.repeat(np.arange(nw), P)            # window of each col
            for w in range(nw):
                comp = 3 + (w % 16)                         # bank-local component
                mbr_v[comp // 2, tt, comp % 2, wincol == w] = -big8
            # lhsT components: 0: 16, 1: 1, 2: BIG, 3+w: BIG iff row's
            # window == w (per bank: component 3+wl maps window b*16+wl)
            w_r = (tcs[t * 128 : (t + 1) * 128] - bs).astype(np.int64)
            assert w_r.min() >= 0 and w_r.max() < nw
            mbl_v[0, tt, :, 0, :] = np.float32(16.0).astype(np_k)
            mbl_v[0, tt, :, 1, :] = np.float32(1.0).astype(np_k)
            mbl_v[1, tt, :, 0, :] = big8
            for bk in range(nb):
                for wl in range(16):
                    comp = 3 + wl
                    sel = w_r == bk * 16 + wl
                    mbl_v[comp // 2, tt, bk, comp % 2, sel] = big8

        in_maps.append(
            {
                "xa": xa_i,
                "cg": np.ascontiguousarray(cg_i.reshape(128, -1)),
                "mb": np.ascontiguousarray(mb_i),
            }
        )
    return nb, nwc, u32, in_maps


def _finish(results):
    s = 0.0
    for r in results:
        o = r["out"].astype(np.float64)
        s += o[:, 0].sum() / 4.0 + o[:, 1].sum() + o[:, 2].sum()
    t = np.float32(s / (B * F))
    ans = np.float32(ALPHA) * t + np.float32(BETA) * (np.float32(1.0) - t)
    return np.asarray(ans, dtype=np.float32)


def kernel(outputs, clusters, target_classes, _run_kwargs=None):
    nb, nwc, u32, in_maps = _prep_inputs(outputs, clusters, target_classes)
    nc = _build_program(nb, nwc, u32)
    kw = _run_kwargs or {}
    res = run_bass_kernel_spmd(nc, in_maps, list(range(NCORES)), **kw)
    ans = _finish(res.results)
    if _run_kwargs is not None:
        kernel.last_result = res
    return ans


if __name__ == "__main__":
    rng = np.random.default_rng(0)
    o = rng.standard_normal((B, F), dtype=np.float32)
    cl = rng.standard_normal((C, P, F), dtype=np.float32)
    t = rng.integers(0, C, size=(B,)).astype(np.int32)
    print(kernel(o, cl, t))
